# revision 5
# baseline (speedup 1.0000x reference)
"""Trainium2 Bass kernel for nn_ContrastiveLoss (in-batch-negatives contrastive loss).

Strategy
--------
Data-parallel over batch B=512: 8 NeuronCores x 64 samples. The reference only
uses the *diagonal* of the in-batch cos_sim matrix, so no all-gather is needed;
each core is fully independent and the final mean is a host-side fold of tiny
per-core partial results.

Per core, the dominant work is streaming irrelevant_passage (32 MB) once from
HBM. The device computes only the big reductions, in a (parity, batch)-packed
layout [p = (n%2)*64 + b, free = d]:

  raw[b,n]  = sum_d src[b,d] * irr[b,n,d]     (unnormalized dot)
  ss[b,n]   = sum_d irr[b,n,d]^2              (squared norms)

via, per [128, 1024] slice:
  DVE : bf16 tensor_mul (2x mode) -> fold-add halves -> (3 of 4 slices)
        tensor_reduce to fp32 column
  ACT : Square + accum_out (fp32) for ss; Copy + accum_out reduce for the
        remaining 1 of 4 slices
Inputs are cast fp32->bf16 for free inside the SWDGE DMA load. src/target stats
(dot, sumsq) are computed in fp32. Normalization, exp, log, and the mean are
done on host in float64 over 66K values (microseconds, exact).

DMA is the roofline: ~36.5 MB/core at ~358 GB/s => ~102 us floor; DVE/ACT are
balanced just under it.
"""

import numpy as np

import concourse.bass as bass
import concourse.mybir as mybir
import concourse.tile as tile
from concourse.bass_utils import run_bass_kernel_spmd

F32 = mybir.dt.float32
BF16 = mybir.dt.bfloat16
ALPHA = 0.8
B, D, P_REL, N_IRR = 512, 1024, 16, 128
NCORES = 8
BL = B // NCORES  # 64 samples per core

# output packing (fp32 [128, 148]):
# cols 0:64 raw_neg, 64:128 ss_neg, 128:136 raw_pos, 136:144 ss_pos,
# col 144 src.tgt dot (parts 0:64), col 145 ss_src (0:64), col 146 ss_tgt (0:64)
OUT_COLS = 148


def _split_excess_waits(nc, max_waits=1):
    """This container's walrus rejects instructions carrying more than
    `max_waits` SyncWaits (the TileContext tail drain accumulates several).
    Splice NOPs on the same engine, each carrying a chunk of the waits."""
    import concourse.mybir as mb

    for bb in nc.main_func.blocks:
        while True:
            insts = list(bb.instructions)
            tgt_idx = None
            for i, ins in enumerate(insts):
                si = ins.sync_info
                if si and si.on_wait and len(si.on_wait) > max_waits:
                    tgt_idx = i
                    break
            if tgt_idx is None:
                break
            ins = insts[tgt_idx]
            w = list(ins.sync_info.on_wait)
            keep, extra = w[:max_waits], w[max_waits:]
            nops = []
            for j in range(0, len(extra), max_waits):
                chunk = extra[j : j + max_waits]
                bnop = nc.engines[ins.engine].nop(nofuse=True)
                nop_inst = None
                for bb2 in nc.main_func.blocks:
                    l2 = list(bb2.instructions)
                    for k, cand in enumerate(l2):
                        if cand.name == bnop.ins.name:
                            nop_inst = cand
                            del l2[k]
                            bb2.instructions = l2
                            break
                    if nop_inst is not None:
                        break
                assert nop_inst is not None
                nop_inst.sync_info = mb.SyncInfo(on_wait=chunk, on_update=[])
                nops.append(nop_inst)
            ins.sync_info = mb.SyncInfo(on_wait=keep, on_update=ins.sync_info.on_update)
            insts = list(bb.instructions)
            tgt_idx = next(i for i, x in enumerate(insts) if x.name == ins.name)
            bb.instructions = insts[:tgt_idx] + nops + insts[tgt_idx:]


def _halves_ap(dram_handle, n_total, m0, mn):
    """AP over [BL, n_total, D] fp32 DRAM enumerating (h:2, b:BL, m:mn, d:D)
    with n = h*(n_total//2) + m0 + m -- matches an SBUF tile [128, mn*D] whose
    partition p = h*64 + b. Free dim (m, d) is contiguous in DRAM, so this is
    a clean 3-dim AP with mn*4KB contiguous runs per partition."""
    full = dram_handle[:, :, :]
    return bass.AP(
        tensor=full.tensor,
        offset=full.offset + m0 * D,
        ap=[[(n_total // 2) * D, 2], [n_total * D, BL], [1, mn * D]],
    )


def _build_program():
    nc = bass.Bass()
    src = nc.dram_tensor("src", [BL, D], F32, kind="ExternalInput")
    tgt = nc.dram_tensor("tgt", [BL, D], F32, kind="ExternalInput")
    rel = nc.dram_tensor("rel", [BL, P_REL, D], F32, kind="ExternalInput")
    irr = nc.dram_tensor("irr", [BL, N_IRR, D], F32, kind="ExternalInput")
    out = nc.dram_tensor("out", [128, OUT_COLS], F32, kind="ExternalOutput")

    JCHUNK = 8  # j's per DMA chunk: [128, 8*1024] bf16 tile, 4 MB fp32 read
    n_irr_chunks = (N_IRR // 2) // JCHUNK  # 8
    Copy = mybir.ActivationFunctionType.Copy
    Square = mybir.ActivationFunctionType.Square
    Exp = mybir.ActivationFunctionType.Exp  # noqa: F841 (host does exp)

    with tile.TileContext(nc) as tc:
        with (
            tc.tile_pool(name="chunks", bufs=3) as chunks,
            tc.tile_pool(name="work", bufs=4) as work,
            tc.tile_pool(name="persist", bufs=1) as persist,
        ):
            # --- small fp32 loads: src, tgt ---
            src_f = persist.tile([BL, D], F32)
            nc.sync.dma_start(out=src_f[:, :], in_=src[:, :])
            tgt_f = persist.tile([BL, D], F32)
            nc.sync.dma_start(out=tgt_f[:, :], in_=tgt[:, :])

            # src duplicated on both partition halves, cast to bf16
            src_dup = persist.tile([128, D], BF16)
            nc.vector.tensor_copy(src_dup[0:BL, :], src_f[:, :])
            nc.vector.tensor_copy(src_dup[BL:128, :], src_f[:, :])

            # --- src/tgt statistics in fp32 ---
            stats = persist.tile([BL, 4], F32)  # cols: st_dot, ss_src, ss_tgt
            prod_st = work.tile([BL, D], F32, tag="prodst")
            nc.vector.tensor_mul(prod_st[:, :], src_f[:, :], tgt_f[:, :])
            nc.vector.tensor_reduce(
                stats[:, 0:1], prod_st[:, :], axis=mybir.AxisListType.X,
                op=mybir.AluOpType.add,
            )
            dummy_act = persist.tile([128, 1], F32)
            nc.scalar.activation(
                dummy_act[0:BL, 0:1].broadcast_to((BL, D)), src_f[:, :],
                Square, accum_out=stats[:, 1:2],
            )
            nc.scalar.activation(
                dummy_act[0:BL, 0:1].broadcast_to((BL, D)), tgt_f[:, :],
                Square, accum_out=stats[:, 2:3],
            )

            # --- accumulator columns ---
            raw_neg = persist.tile([128, N_IRR // 2], F32)
            ss_neg = persist.tile([128, N_IRR // 2], F32)
            raw_pos = persist.tile([128, P_REL // 2], F32)
            ss_pos = persist.tile([128, P_REL // 2], F32)

            # --- main streaming loop ---
            # chunk list: (dram, n_total, j0, raw_buf, ss_buf, col0)
            chunk_list = [
                (irr, N_IRR, k * JCHUNK, raw_neg, ss_neg, k * JCHUNK)
                for k in range(n_irr_chunks)
            ]
            chunk_list.append((rel, P_REL, 0, raw_pos, ss_pos, 0))

            gslice = 0
            for dram, n_total, j0, raw_buf, ss_buf, col0 in chunk_list:
                ctile = chunks.tile([128, JCHUNK * D], BF16, tag="chunk")
                # SWDGE load with free fp32->bf16 cast
                nc.gpsimd.dma_start(
                    out=ctile[:, :], in_=_halves_ap(dram, n_total, j0, JCHUNK)
                )
                for j in range(JCHUNK):
                    sl = ctile[:, j * D : (j + 1) * D]
                    col = col0 + j
                    # ACT: sum of squares (bf16 in, fp32 accum)
                    nc.scalar.activation(
                        dummy_act.broadcast_to((128, D)), sl,
                        Square, accum_out=ss_buf[:, col : col + 1],
                    )
                    # DVE: products (bf16 2x), fold halves
                    prod = work.tile([128, D], BF16, tag="prod")
                    nc.vector.tensor_mul(prod[:, :], sl, src_dup[:, :])
                    fold = work.tile([128, D // 2], BF16, tag="fold")
                    nc.vector.tensor_add(
                        fold[:, :], prod[:, 0 : D // 2], prod[:, D // 2 : D]
                    )
                    if gslice % 4 != 3:
                        nc.vector.tensor_reduce(
                            raw_buf[:, col : col + 1], fold[:, :],
                            axis=mybir.AxisListType.X, op=mybir.AluOpType.add,
                        )
                    else:
                        nc.scalar.activation(
                            dummy_act.broadcast_to((128, D // 2)), fold[:, :],
                            Copy, accum_out=raw_buf[:, col : col + 1],
                        )
                    gslice += 1

            # --- write outputs ---
            nc.sync.dma_start(out=out[:, 0:64], in_=raw_neg[:, :])
            nc.sync.dma_start(out=out[:, 64:128], in_=ss_neg[:, :])
            nc.sync.dma_start(out=out[:, 128:136], in_=raw_pos[:, :])
            nc.sync.dma_start(out=out[:, 136:144], in_=ss_pos[:, :])
            nc.sync.dma_start(out=out[0:BL, 144:147], in_=stats[:, 0:3])

    _split_excess_waits(nc, max_waits=1)
    return nc


_NC_CACHE = None


def _get_nc():
    global _NC_CACHE
    if _NC_CACHE is None:
        _NC_CACHE = _build_program()
    return _NC_CACHE


def _run_device(in_maps, trace=False, **kw):
    nc = _get_nc()
    return run_bass_kernel_spmd(
        nc, in_maps, core_ids=list(range(NCORES)), trace=trace, **kw
    )


def make_in_maps(embeddings_src, embeddings_target, relevant_passage, irrelevant_passage):
    in_maps = []
    for c in range(NCORES):
        sl = slice(c * BL, (c + 1) * BL)
        in_maps.append(
            {
                "src": np.ascontiguousarray(embeddings_src[sl]),
                "tgt": np.ascontiguousarray(embeddings_target[sl]),
                "rel": np.ascontiguousarray(relevant_passage[sl]),
                "irr": np.ascontiguousarray(irrelevant_passage[sl]),
            }
        )
    return in_maps


def finish_on_host(core_outs):
    """core_outs: list of NCORES arrays [128, OUT_COLS] fp32 -> scalar loss."""
    raw_neg = np.empty((B, N_IRR), np.float64)
    ss_neg = np.empty((B, N_IRR), np.float64)
    raw_pos = np.empty((B, P_REL), np.float64)
    ss_pos = np.empty((B, P_REL), np.float64)
    st_dot = np.empty((B,), np.float64)
    ss_src = np.empty((B,), np.float64)
    ss_tgt = np.empty((B,), np.float64)
    for c, o in enumerate(core_outs):
        o = o.astype(np.float64)
        bsl = slice(c * BL, (c + 1) * BL)
        for h in range(2):
            rows = slice(h * BL, (h + 1) * BL)
            raw_neg[bsl, h * 64 : (h + 1) * 64] = o[rows, 0:64]
            ss_neg[bsl, h * 64 : (h + 1) * 64] = o[rows, 64:128]
            raw_pos[bsl, h * 8 : (h + 1) * 8] = o[rows, 128:136]
            ss_pos[bsl, h * 8 : (h + 1) * 8] = o[rows, 136:144]
        st_dot[bsl] = o[0:BL, 144]
        ss_src[bsl] = o[0:BL, 145]
        ss_tgt[bsl] = o[0:BL, 146]

    nrm_s = np.sqrt(np.clip(ss_src, 1e-24, None))
    diag = st_dot / np.clip(nrm_s * np.sqrt(ss_tgt), 1e-12, None)
    pos_sims = raw_pos / np.clip(nrm_s[:, None] * np.sqrt(ss_pos), 1e-12, None)
    neg_sims = raw_neg / np.clip(nrm_s[:, None] * np.sqrt(ss_neg), 1e-12, None)
    pos_score = 1.0 + np.exp(pos_sims).sum(axis=1)
    neg_score = np.exp(neg_sims).sum(axis=1)
    loss_pos = np.log(pos_score)
    loss_neg = np.log(pos_score + neg_score)
    loss = np.mean(-(ALPHA * diag + (1.0 - ALPHA) * (loss_pos - loss_neg)))
    return np.float32(loss)


def kernel(embeddings_src, embeddings_target, relevant_passage, irrelevant_passage):
    in_maps = make_in_maps(
        embeddings_src, embeddings_target, relevant_passage, irrelevant_passage
    )
    res = _run_device(in_maps)
    return np.asarray(
        finish_on_host([res.results[c]["out"] for c in range(NCORES)]),
        dtype=np.float32,
    )


# revision 9
# speedup vs baseline: 2.3493x; 2.3493x over previous
"""Trainium2 Bass kernel for nn_ContrastiveLoss (in-batch-negatives contrastive loss).

Strategy
--------
Data-parallel over batch B=512: 8 NeuronCores x 64 samples. The reference only
uses the *diagonal* of the in-batch cos_sim matrix, so no all-gather is needed;
each core is fully independent and the final mean is a host-side fold of tiny
per-core partial results.

Per core, the dominant work is streaming irrelevant_passage (32 MB) once from
HBM. The device computes only the big reductions, in a (parity, batch)-packed
layout [p = (n%2)*64 + b, free = d]:

  raw[b,n]  = sum_d src[b,d] * irr[b,n,d]     (unnormalized dot)
  ss[b,n]   = sum_d irr[b,n,d]^2              (squared norms)

via, per [128, 1024] slice:
  DVE : bf16 tensor_mul (2x mode) -> fold-add halves -> (3 of 4 slices)
        tensor_reduce to fp32 column
  ACT : Square + accum_out (fp32) for ss; Copy + accum_out reduce for the
        remaining 1 of 4 slices
Inputs are cast fp32->bf16 for free inside the SWDGE DMA load. src/target stats
(dot, sumsq) are computed in fp32. Normalization, exp, log, and the mean are
done on host in float64 over 66K values (microseconds, exact).

DMA is the roofline: ~36.5 MB/core at ~358 GB/s => ~102 us floor; DVE/ACT are
balanced just under it.
"""

import numpy as np

import concourse.bass as bass
import concourse.mybir as mybir
import concourse.tile as tile
from concourse.bass_utils import run_bass_kernel_spmd

F32 = mybir.dt.float32
BF16 = mybir.dt.bfloat16
ALPHA = 0.8
B, D, P_REL, N_IRR = 512, 1024, 16, 128
NCORES = 8
BL = B // NCORES  # 64 samples per core

# output packing (fp32 [128, 148]):
# cols 0:64 raw_neg, 64:128 ss_neg, 128:136 raw_pos, 136:144 ss_pos,
# col 144 src.tgt dot (parts 0:64), col 145 ss_src (0:64), col 146 ss_tgt (0:64)
OUT_COLS = 148


def _split_excess_waits(nc, max_waits=1):
    """This container's walrus rejects instructions carrying more than
    `max_waits` SyncWaits (the TileContext tail drain accumulates several).
    Splice NOPs on the same engine, each carrying a chunk of the waits."""
    import concourse.mybir as mb

    for bb in nc.main_func.blocks:
        while True:
            insts = list(bb.instructions)
            tgt_idx = None
            for i, ins in enumerate(insts):
                si = ins.sync_info
                if si and si.on_wait and len(si.on_wait) > max_waits:
                    tgt_idx = i
                    break
            if tgt_idx is None:
                break
            ins = insts[tgt_idx]
            w = list(ins.sync_info.on_wait)
            keep, extra = w[:max_waits], w[max_waits:]
            nops = []
            for j in range(0, len(extra), max_waits):
                chunk = extra[j : j + max_waits]
                bnop = nc.engines[ins.engine].nop(nofuse=True)
                nop_inst = None
                for bb2 in nc.main_func.blocks:
                    l2 = list(bb2.instructions)
                    for k, cand in enumerate(l2):
                        if cand.name == bnop.ins.name:
                            nop_inst = cand
                            del l2[k]
                            bb2.instructions = l2
                            break
                    if nop_inst is not None:
                        break
                assert nop_inst is not None
                nop_inst.sync_info = mb.SyncInfo(on_wait=chunk, on_update=[])
                nops.append(nop_inst)
            ins.sync_info = mb.SyncInfo(on_wait=keep, on_update=ins.sync_info.on_update)
            insts = list(bb.instructions)
            tgt_idx = next(i for i, x in enumerate(insts) if x.name == ins.name)
            bb.instructions = insts[:tgt_idx] + nops + insts[tgt_idx:]


# The passage tensors are pre-shuffled on host to [128, (n_total//2)*D] with
# partition p = h*64 + b holding rows n = h*(n_total//2) + m, so every device
# DMA is a contiguous uniform-stride slab (measured 372 GB/s vs 125 GB/s for
# the equivalent strided 3-level AP).


def _build_program():
    nc = bass.Bass()
    src = nc.dram_tensor("src", [BL, D], F32, kind="ExternalInput")
    tgt = nc.dram_tensor("tgt", [BL, D], F32, kind="ExternalInput")
    rel = nc.dram_tensor("rel", [128, (P_REL // 2) * D], F32, kind="ExternalInput")
    irr = nc.dram_tensor("irr", [128, (N_IRR // 2) * D], F32, kind="ExternalInput")
    out = nc.dram_tensor("out", [128, OUT_COLS], F32, kind="ExternalOutput")

    JCHUNK = 8  # j's per DMA chunk: [128, 8*1024] bf16 tile, 4 MB fp32 read
    n_irr_chunks = (N_IRR // 2) // JCHUNK  # 8
    Copy = mybir.ActivationFunctionType.Copy
    Square = mybir.ActivationFunctionType.Square
    Exp = mybir.ActivationFunctionType.Exp  # noqa: F841 (host does exp)

    with tile.TileContext(nc) as tc:
        with (
            tc.tile_pool(name="chunks", bufs=3) as chunks,
            tc.tile_pool(name="work", bufs=4) as work,
            tc.tile_pool(name="persist", bufs=1) as persist,
        ):
            # --- small fp32 loads: src, tgt ---
            src_f = persist.tile([BL, D], F32)
            nc.sync.dma_start(out=src_f[:, :], in_=src[:, :])
            tgt_f = persist.tile([BL, D], F32)
            nc.sync.dma_start(out=tgt_f[:, :], in_=tgt[:, :])

            # src duplicated on both partition halves, cast to bf16
            src_dup = persist.tile([128, D], BF16)
            nc.vector.tensor_copy(src_dup[0:BL, :], src_f[:, :])
            nc.vector.tensor_copy(src_dup[BL:128, :], src_f[:, :])

            # --- src/tgt statistics in fp32 ---
            stats = persist.tile([BL, 4], F32)  # cols: st_dot, ss_src, ss_tgt
            prod_st = work.tile([BL, D], F32, tag="prodst")
            nc.vector.tensor_mul(prod_st[:, :], src_f[:, :], tgt_f[:, :])
            nc.vector.tensor_reduce(
                stats[:, 0:1], prod_st[:, :], axis=mybir.AxisListType.X,
                op=mybir.AluOpType.add,
            )
            dummy_act = persist.tile([128, 1], F32)
            nc.scalar.activation(
                dummy_act[0:BL, 0:1].broadcast_to((BL, D)), src_f[:, :],
                Square, accum_out=stats[:, 1:2],
            )
            nc.scalar.activation(
                dummy_act[0:BL, 0:1].broadcast_to((BL, D)), tgt_f[:, :],
                Square, accum_out=stats[:, 2:3],
            )

            # --- accumulator columns ---
            raw_neg = persist.tile([128, N_IRR // 2], F32)
            ss_neg = persist.tile([128, N_IRR // 2], F32)
            raw_pos = persist.tile([128, P_REL // 2], F32)
            ss_pos = persist.tile([128, P_REL // 2], F32)

            # --- main streaming loop ---
            # chunk list: (dram, j0, raw_buf, ss_buf, col0)
            chunk_list = [
                (irr, k * JCHUNK, raw_neg, ss_neg, k * JCHUNK)
                for k in range(n_irr_chunks)
            ]
            chunk_list.append((rel, 0, raw_pos, ss_pos, 0))

            gslice = 0
            for dram, j0, raw_buf, ss_buf, col0 in chunk_list:
                ctile = chunks.tile([128, JCHUNK * D], BF16, tag="chunk")
                # SWDGE load (contiguous slab) with free fp32->bf16 cast
                nc.gpsimd.dma_start(
                    out=ctile[:, :],
                    in_=dram[:, j0 * D : (j0 + JCHUNK) * D],
                )
                for j in range(JCHUNK):
                    sl = ctile[:, j * D : (j + 1) * D]
                    col = col0 + j
                    # ACT: sum of squares (bf16 in, fp32 accum)
                    nc.scalar.activation(
                        dummy_act.broadcast_to((128, D)), sl,
                        Square, accum_out=ss_buf[:, col : col + 1],
                    )
                    # DVE: products (bf16 2x), fold halves
                    prod = work.tile([128, D], BF16, tag="prod")
                    nc.vector.tensor_mul(prod[:, :], sl, src_dup[:, :])
                    fold = work.tile([128, D // 2], BF16, tag="fold")
                    nc.vector.tensor_add(
                        fold[:, :], prod[:, 0 : D // 2], prod[:, D // 2 : D]
                    )
                    if gslice % 4 != 3:
                        nc.vector.tensor_reduce(
                            raw_buf[:, col : col + 1], fold[:, :],
                            axis=mybir.AxisListType.X, op=mybir.AluOpType.add,
                        )
                    else:
                        nc.scalar.activation(
                            dummy_act.broadcast_to((128, D // 2)), fold[:, :],
                            Copy, accum_out=raw_buf[:, col : col + 1],
                        )
                    gslice += 1

            # --- write outputs ---
            nc.sync.dma_start(out=out[:, 0:64], in_=raw_neg[:, :])
            nc.sync.dma_start(out=out[:, 64:128], in_=ss_neg[:, :])
            nc.sync.dma_start(out=out[:, 128:136], in_=raw_pos[:, :])
            nc.sync.dma_start(out=out[:, 136:144], in_=ss_pos[:, :])
            nc.sync.dma_start(out=out[0:BL, 144:147], in_=stats[:, 0:3])

    _split_excess_waits(nc, max_waits=1)
    return nc


_NC_CACHE = None


def _get_nc():
    global _NC_CACHE
    if _NC_CACHE is None:
        _NC_CACHE = _build_program()
    return _NC_CACHE


def _run_device(in_maps, trace=False, **kw):
    nc = _get_nc()
    return run_bass_kernel_spmd(
        nc, in_maps, core_ids=list(range(NCORES)), trace=trace, **kw
    )


def make_in_maps(embeddings_src, embeddings_target, relevant_passage, irrelevant_passage):
    in_maps = []
    for c in range(NCORES):
        sl = slice(c * BL, (c + 1) * BL)
        rel_dev = np.ascontiguousarray(
            np.asarray(relevant_passage[sl])
            .reshape(BL, 2, (P_REL // 2) * D)
            .transpose(1, 0, 2)
        ).reshape(128, (P_REL // 2) * D)
        irr_dev = np.ascontiguousarray(
            np.asarray(irrelevant_passage[sl])
            .reshape(BL, 2, (N_IRR // 2) * D)
            .transpose(1, 0, 2)
        ).reshape(128, (N_IRR // 2) * D)
        in_maps.append(
            {
                "src": np.ascontiguousarray(embeddings_src[sl]),
                "tgt": np.ascontiguousarray(embeddings_target[sl]),
                "rel": rel_dev,
                "irr": irr_dev,
            }
        )
    return in_maps


def finish_on_host(core_outs):
    """core_outs: list of NCORES arrays [128, OUT_COLS] fp32 -> scalar loss."""
    raw_neg = np.empty((B, N_IRR), np.float64)
    ss_neg = np.empty((B, N_IRR), np.float64)
    raw_pos = np.empty((B, P_REL), np.float64)
    ss_pos = np.empty((B, P_REL), np.float64)
    st_dot = np.empty((B,), np.float64)
    ss_src = np.empty((B,), np.float64)
    ss_tgt = np.empty((B,), np.float64)
    for c, o in enumerate(core_outs):
        o = o.astype(np.float64)
        bsl = slice(c * BL, (c + 1) * BL)
        for h in range(2):
            rows = slice(h * BL, (h + 1) * BL)
            raw_neg[bsl, h * 64 : (h + 1) * 64] = o[rows, 0:64]
            ss_neg[bsl, h * 64 : (h + 1) * 64] = o[rows, 64:128]
            raw_pos[bsl, h * 8 : (h + 1) * 8] = o[rows, 128:136]
            ss_pos[bsl, h * 8 : (h + 1) * 8] = o[rows, 136:144]
        st_dot[bsl] = o[0:BL, 144]
        ss_src[bsl] = o[0:BL, 145]
        ss_tgt[bsl] = o[0:BL, 146]

    nrm_s = np.sqrt(np.clip(ss_src, 1e-24, None))
    diag = st_dot / np.clip(nrm_s * np.sqrt(ss_tgt), 1e-12, None)
    pos_sims = raw_pos / np.clip(nrm_s[:, None] * np.sqrt(ss_pos), 1e-12, None)
    neg_sims = raw_neg / np.clip(nrm_s[:, None] * np.sqrt(ss_neg), 1e-12, None)
    pos_score = 1.0 + np.exp(pos_sims).sum(axis=1)
    neg_score = np.exp(neg_sims).sum(axis=1)
    loss_pos = np.log(pos_score)
    loss_neg = np.log(pos_score + neg_score)
    loss = np.mean(-(ALPHA * diag + (1.0 - ALPHA) * (loss_pos - loss_neg)))
    return np.float32(loss)


def kernel(embeddings_src, embeddings_target, relevant_passage, irrelevant_passage):
    in_maps = make_in_maps(
        embeddings_src, embeddings_target, relevant_passage, irrelevant_passage
    )
    res = _run_device(in_maps)
    return np.asarray(
        finish_on_host([res.results[c]["out"] for c in range(NCORES)]),
        dtype=np.float32,
    )
